# revision 1
# baseline (speedup 1.0000x reference)
"""Trainium2 Bass kernel for nn_AbstractODEDecoder.

Data-parallel over batch across 8 cores. Each core:
  - integrates the 100-step dopri5 ODE for its 64-row batch shard
    (fp32, feature-major, fully unrolled),
  - spills v_all rows to DRAM batch-major per step,
  - decodes 12800 (b, n) rows sorted by time index, gathering
    latent+z_static rows via dma_gather (float32r matmuls, N=400),
  - interleaved with the ODE via AP-granular DRAM deps.
Host: unique-time grid, per-stage effective biases, gather indices,
input sharding / output assembly.
"""
import numpy as np

import concourse.bass as bass
import concourse.mybir as mybir
import concourse.tile as tile
from concourse import bacc
from concourse.masks import make_identity

F32 = mybir.dt.float32
F32R = mybir.dt.float32r
BF16DT = mybir.dt.bfloat16
I16 = mybir.dt.int16
TANH = mybir.ActivationFunctionType.Tanh
RELU = mybir.ActivationFunctionType.Relu
ADD = mybir.AluOpType.add
MULT = mybir.AluOpType.mult
MAX = mybir.AluOpType.max

B, N, ZD, LD, HD, T = 512, 200, 256, 128, 512, 100
NST, NCORE, BL = 100, 8, 64
import os as _os0
RT = int(_os0.environ.get('KRT', '400'))
ROWS, NTILE = BL * N, BL * N // RT
import os as _os
NH = int(_os.environ.get('KNH', '2'))   # batch-halves pipelined per core
NB = BL // NH             # batch per half
SKIP_DECODE = _os.environ.get('KSKIPDEC', '') == '1'
SKIP_ODE = _os.environ.get('KSKIPODE', '') == '1'
XNOCB = _os.environ.get('KNOCB', '') == '1'        # timing expt: drop bias adds
XNOCOMBO = _os.environ.get('KNOCOMBO', '') == '1'  # timing expt: drop combos
XACTDVE = _os.environ.get('KACTDVE', '') == '1'    # timing expt: tanh->copy
XNOL2 = _os.environ.get('KNOL2', '') == '1'        # timing expt: 4 L2 MMs
XNOB1DMA = _os.environ.get('KNOB1DMA', '') == '1'  # timing expt: b1 lhsT = b2mm
XNOBIASMM = _os.environ.get('KNOBIASMM', '') == '1'  # timing expt: drop bias MMs
POOLCOMBO = _os.environ.get('KPOOLCOMBO', '0') == '1'  # partials on gpsimd
DRIP = int(_os.environ.get('KDRIP', '1'))           # decode pieces per stage slot
DRIPFINE = _os.environ.get('KDRIPFINE', '0') == '1'  # feed after each half too
HORDER = _os.environ.get('KHORDER', '0') == '1'      # stage+partials per half
STEPORD = _os.environ.get('KSTEPORD', '0') == '1'    # whole step per half
DRIPPRE = _os.environ.get('KDRIPPRE', '0') == '1'    # feed before stage ops
OMERGE = _os.environ.get('KOMERGE', '0') == '1'      # merged decode out DMA
KEVAC = _os.environ.get('KKEVAC', '1') == '1'        # khat->SBUF, partials on Pool
DVERELU = _os.environ.get('KDVERELU', '0') == '1'   # decode relu on DVE
ACTSPLIT = _os.environ.get('KACTSPLIT', '0') == '1'  # tanh as 2 half ACTs
_bf = _os.environ.get('KBF16', '0')
BF16 = _bf in ('1', '2')            # bf16 L2 (2: L2 only, 1: L2+L3)
BF16L3 = _bf == '1'

C_ = [0.0, 1.0 / 5.0, 3.0 / 10.0, 4.0 / 5.0, 8.0 / 9.0, 1.0]
A_ = [[],
      [1.0 / 5.0],
      [3.0 / 40.0, 9.0 / 40.0],
      [44.0 / 45.0, -56.0 / 15.0, 32.0 / 9.0],
      [19372.0 / 6561.0, -25360.0 / 2187.0, 64448.0 / 6561.0, -212.0 / 729.0],
      [9017.0 / 3168.0, -355.0 / 33.0, 46732.0 / 5247.0, 49.0 / 176.0,
       -5103.0 / 18656.0]]
B_ = [35.0 / 384.0, 0.0, 500.0 / 1113.0, 125.0 / 192.0, -2187.0 / 6784.0,
      11.0 / 84.0]

_BUILD_CACHE = {}


def _build(maxinds, dts):
    """Build + bacc-compile the SPMD program. maxinds: per-decode-tile max
    time index (same for every core by construction of the schedule)."""
    VMAP, RATIO, RATIOU = _build.vmap, _build.ratio, _build.ratiou
    key = ('v12', KEVAC, OMERGE, DRIPPRE, STEPORD, DRIPFINE, HORDER, DRIP, _os.environ.get('KDTR','0'), _os.environ.get('KDECBUFS','2'), DVERELU, ACTSPLIT, RT, _os.environ.get('KODEBUFS','3'), BF16L3, _os.environ.get('KB1DVE','0'), _os.environ.get('KDLBUFS','2'), NH, SKIP_DECODE, SKIP_ODE, XNOCB, XNOCOMBO, XACTDVE, XNOL2, XNOB1DMA, XNOBIASMM, POOLCOMBO, BF16, tuple(maxinds), tuple(np.float32(dts).tolist()))
    if key in _BUILD_CACHE:
        return _BUILD_CACHE[key]

    nc = bacc.Bacc('TRN2', target_bir_lowering=False, debug=False,
                   num_devices=NCORE, num_swdge_queues=4)
    dram = {}

    def din(name, shape, dt):
        dram[name] = nc.dram_tensor(name, shape, dt, kind='ExternalInput').ap()
        return dram[name]

    # ODE weights (fp32) in lhsT tile layouts
    w1l_d = din('w1l', [128, 4, 128], F32)        # [p, m, f] = W1[p, m*128+f]
    wdt = BF16DT if BF16 else F32
    wdt3 = BF16DT if BF16L3 else F32
    w2_d = din('w2', [128, 4, 4, 128], wdt)       # [p, k, m, f]
    NV = _build.nv
    w3v_d = din('w3v', [128, NV, 4, 128], wdt3)   # dt*coef prescaled W3
    c1bm_d = din('c1bm', [64, 512], F32)          # z@W1Z batch-major
    b1mm_d = din('b1mm', [NST * 6, 4, 512], F32)  # per-stage bias rows (4x)
    b2mm_d = din('b2mm', [4, 128], F32)
    onesid_d = din('onesid', [68, 4, BL], F32)    # [I64; e_c] selector
    onesel_d = din('onesel', [4, 4, BL], F32)     # e_c x ones selector
    db3_d = din('db3', [128, NST], F32)
    vlt0_d = din('vlt0', [128, BL], F32)
    zl_d = din('zl', [BL, 128], F32)              # z latent, batch-major
    zz_d = din('zz', [BL, 128], F32)              # z static, batch-major
    # decode weights (float32r)
    d1l_d = din('d1l', [128, 4, 128], F32R)
    d1z_d = din('d1z', [128, 4, 128], F32R)
    d1x_d = din('d1x', [1, 512], F32R)
    d2_d = din('d2', [128, 4, 4, 128], F32R)
    d3_d = din('d3', [128, 4, 4, 128], F32R)
    dbias_d = din('dbias', [128, 4, 3], F32)
    xs_d = din('xs', [1, ROWS], F32R)
    gidx_d = din('gidx', [128, NTILE * 32], I16)  # idx cols per tile, pad 32

    vall_d = nc.dram_tensor('vall', [(NST + 1) * BL, 256], F32).ap()
    out_d = nc.dram_tensor('outT', [4, 128, ROWS], F32,
                           kind='ExternalOutput').ap()

    with tile.TileContext(nc) as tc, \
         tc.tile_pool(name='sing', bufs=1) as sing, \
         tc.tile_pool(name='ode', bufs=int(_os.environ.get('KODEBUFS','3'))) as ode, \
         tc.tile_pool(name='dec', bufs=int(_os.environ.get('KDECBUFS','2'))) as dec, \
         tc.tile_pool(name='psA', bufs=1, space='PSUM') as psA, \
         tc.tile_pool(name='psD', bufs=int(_os.environ.get('KDLBUFS','3')), space='PSUM') as psD:

        # ---- residents ----
        w1l = sing.tile([128, 4, 128], F32)
        nc.sync.dma_start(out=w1l, in_=w1l_d)
        w2 = sing.tile([128, 4, 4, 128], wdt)
        nc.sync.dma_start(out=w2, in_=w2_d)
        w3v = sing.tile([128, NV, 4, 128], wdt3)
        nc.sync.dma_start(out=w3v, in_=w3v_d)
        c1b = [sing.tile([68, 512], F32, name=f'c1b{i}') for i in range(4)]
        for i in range(4):
            nc.sync.dma_start(out=c1b[i][0:64, :], in_=c1bm_d)
        onesid = sing.tile([68, 4, BL], F32)
        nc.sync.dma_start(out=onesid, in_=onesid_d)
        b2mm = sing.tile([4, 128], F32)
        nc.sync.dma_start(out=b2mm, in_=b2mm_d)
        onesel = sing.tile([4, 4, BL], F32)
        nc.sync.dma_start(out=onesel, in_=onesel_d)
        db3 = sing.tile([128, NST], F32)
        nc.sync.dma_start(out=db3, in_=db3_d)
        d1l = sing.tile([128, 4, 128], F32R)
        nc.sync.dma_start(out=d1l, in_=d1l_d)
        d1z = sing.tile([128, 4, 128], F32R)
        nc.sync.dma_start(out=d1z, in_=d1z_d)
        d1x = sing.tile([1, 512], F32R)
        nc.sync.dma_start(out=d1x, in_=d1x_d)
        d2 = sing.tile([128, 4, 4, 128], F32R)
        nc.sync.dma_start(out=d2, in_=d2_d)
        d3 = sing.tile([128, 4, 4, 128], F32R)
        nc.sync.dma_start(out=d3, in_=d3_d)
        dbias = sing.tile([128, 4, 3], F32)
        nc.sync.dma_start(out=dbias, in_=dbias_d)
        xs = sing.tile([1, ROWS], F32R)
        nc.sync.dma_start(out=xs, in_=xs_d)
        gidx = sing.tile([128, NTILE * 32], I16)
        nc.sync.dma_start(out=gidx, in_=gidx_d)
        ident = sing.tile([128, 128], F32)
        make_identity(nc, ident)

        # ---- v_all init: z_static for every step; latent for step 0 ----
        vall_v = vall_d.rearrange('(s b) f -> s b f', b=BL)
        zz_b = bass.AP(tensor=zz_d.tensor, offset=zz_d.offset,
                       ap=[[0, NST + 1], *zz_d.ap])
        nc.sync.dma_start(out=vall_v[:, :, 128:256], in_=zz_b)
        nc.sync.dma_start(out=vall_v[0, :, 0:128], in_=zl_d)

        # ---- initial v latent per half ----
        vl = []
        for h in range(NH):
            t0 = ode.tile([128, NB], F32, tag=f'vl{h}', name=f'vl0_{h}')
            nc.sync.dma_start(out=t0, in_=vlt0_d[:, h * NB:(h + 1) * NB])
            vl.append(t0)

        def bcast(ap, n):
            return bass.AP(tensor=ap.tensor, offset=ap.offset,
                           ap=[*ap.ap, [0, n]])

        def emit_stage(h, s, i, hkp, kb, part):
            si = s * 6 + i
            h1r = hkp[:, 0:4 * NB]
            h2r = hkp[:, 10 * NB:14 * NB] if NH > 1 else emit_stage.h2sep[h]
            # vtmp (on-chain single add)
            if i == 0 or XNOCOMBO:
                vt = vl[h]
            else:
                pt = vl[h] if i == 1 else part[i]
                vtt = ode.tile([128, NB], F32, tag=f'vt{h}',
                               name=f'vt_{h}_{si}')
                nc.vector.tensor_tensor(vtt, pt, kb(i - 1), ADD)
                vt = vtt
            # L1: bias/z inject (K=68) + vL matmul per chunk
            if h == 0:
                cb = c1b[si % 4]
                (nc.vector if _os.environ.get('KB1DVE','0')=='1' else nc.sync).dma_start(out=cb[64:68, :], in_=b1mm_d[si])
                emit_stage.cb = cb
            cb = emit_stage.cb
            for m in range(4):
                reg = h1r[:, m * NB:(m + 1) * NB]
                nc.tensor.matmul(reg, cb[:, m * 128:(m + 1) * 128],
                                 onesid[:, m, h * NB:h * NB + NB],
                                 start=True, stop=False)
                nc.tensor.matmul(reg, w1l[:, m, :], vt,
                                 start=False, stop=True)
            h1v = h1r.rearrange('p (m j) -> p m j', m=4)
            h1t = ode.tile([128, 4, NB], BF16DT if BF16 else F32,
                           tag=f'h1t{h}', name=f'h1t_{h}_{si}')
            if ACTSPLIT:
                nc.scalar.activation(h1t[:, 0:2, :], h1v[:, 0:2, :], TANH)
                nc.scalar.activation(h1t[:, 2:4, :], h1v[:, 2:4, :], TANH)
            else:
                nc.scalar.activation(h1t, h1v, TANH)
            # L2
            for m in range(4):
                reg = h2r[:, m * NB:(m + 1) * NB]
                nc.tensor.matmul(reg, b2mm,
                                 onesel[:, m, h * NB:h * NB + NB],
                                 start=True, stop=False)
                for k in range(4):
                    nc.tensor.matmul(reg, w2[:, k, m, :], h1t[:, k, :],
                                     start=False, stop=(k == 3))
            h2v = h2r.rearrange('p (m j) -> p m j', m=4)
            h2t = ode.tile([128, 4, NB], BF16DT if BF16L3 else F32,
                           tag=f'h2t{h}', name=f'h2t_{h}_{si}')
            if ACTSPLIT:
                nc.scalar.activation(h2t[:, 0:2, :], h2v[:, 0:2, :], TANH)
                nc.scalar.activation(h2t[:, 2:4, :], h2v[:, 2:4, :], TANH)
            else:
                nc.scalar.activation(h2t, h2v, TANH)
            # L3 -> khat slot i (dt*coef prescaled)
            vi = VMAP[si]
            for k in range(4):
                nc.tensor.matmul(kb(i), w3v[:, vi, k, :],
                                 h2t[:, k, :], start=(k == 0), stop=(k == 3))
            if KEVAC:
                ks = ode.tile([128, NB], F32, tag=f'ks{h}{i}',
                              name=f'ks_{h}_{si}')
                nc.vector.tensor_copy(ks, kb(i))
                emit_stage.kse[h][i] = ks

        def emit_partials(h, s, j, kb, part):
            """After half h's L3_j: push khat_j into future partials."""
            if KEVAC:
                eng = nc.gpsimd
                kb = lambda jj: emit_stage.kse[h][jj]
            else:
                eng = nc.gpsimd if POOLCOMBO else nc.vector
            for i in range(max(j + 2, 2), 6):
                r = RATIO[s][i][j]
                if r == 0.0:
                    continue
                if j == 0:
                    t = ode.tile([128, NB], F32, tag=f'pt{i}{h}',
                                 name=f'pt_{i}_{h}_{s}')
                    eng.tensor_scalar(t, kb(j), r, None, MULT)
                    part[i] = t
                else:
                    tmp = ode.tile([128, NB], F32, tag=f'tm{h}',
                                   name=f'tm_{h}_{s}_{i}_{j}')
                    eng.tensor_scalar(tmp, kb(j), r, None, MULT)
                    eng.tensor_tensor(part[i], part[i], tmp, ADD)
                if j == i - 2:
                    eng.tensor_tensor(part[i], part[i], vl[h], ADD)
            if j <= 4:
                r = RATIOU[s][j]
                if r != 0.0:
                    if j == 0:
                        t = ode.tile([128, NB], F32, tag=f'ptU{h}',
                                     name=f'ptU_{h}_{s}')
                        eng.tensor_scalar(t, kb(j), r,
                                          db3[:, s:s + 1], MULT, ADD)
                        part['U'] = t
                    else:
                        tmp = ode.tile([128, NB], F32, tag=f'tm{h}',
                                       name=f'tmU_{h}_{s}_{j}')
                        eng.tensor_scalar(tmp, kb(j), r, None, MULT)
                        eng.tensor_tensor(part['U'], part['U'], tmp, ADD)
                if j == 4:
                    eng.tensor_tensor(part['U'], part['U'], vl[h], ADD)

        def emit_vupdate(h, s, kb, part):
            vnew = ode.tile([128, NB], F32, tag=f'vl{h}', name=f'vn_{h}_{s}')
            nc.vector.tensor_tensor(vnew, part['U'], kb(5), ADD)
            vl[h] = vnew
            trp = psA.tile([128, 128], F32, tag='tr', name=f'vtr_{h}_{s}')
            nc.tensor.transpose(trp[0:NB, :], vnew, ident)
            vvb = ode.tile([NB, 128], F32, tag=f'vb{h}', name=f'vb_{h}_{s}')
            nc.vector.tensor_copy(vvb, trp[0:NB, :])
            nc.sync.dma_start(
                out=vall_v[s + 1, h * NB:(h + 1) * NB, 0:128], in_=vvb)

        def emit_decode_tile(ti, mi):
            for _ in emit_decode_gen(ti, mi):
                pass

        def emit_decode_gen(ti, mi):
            r0 = ti * RT
            # gather 400 rows of 256 f32 from the written prefix of vall
            g_sb = dec.tile([128, 4, 256], F32, tag='g', name=f'g_{ti}')
            nc.gpsimd.dma_gather(
                g_sb[:], vall_d[0:(mi + 1) * BL], gidx[:, ti * 32:ti * 32 + RT // 16],
                num_idxs=RT, num_idxs_reg=RT, elem_size=256,
                queue_num=ti % 4)
            # transpose to feature-major: latent rows then z rows
            latT = dec.tile([128, RT], F32R, tag='latT', name=f'latT_{ti}')
            zT = dec.tile([128, RT], F32R, tag='zT', name=f'zT_{ti}')
            yield
            for half, dst in ((0, latT), (1, zT)):
                trp = psD.tile([128, 512], F32, tag='dtr', name=f'dtr_{ti}_{half}') \
                    if _os.environ.get('KDTR', '0') == '1' else \
                    psA.tile([128, 512], F32, tag='tr', name=f'dtr_{ti}_{half}')
                for c in range(4):
                    nc.tensor.transpose(
                        trp[:, c * 128:(c + 1) * 128],
                        g_sb[:, c, half * 128:(half + 1) * 128], ident)
                nc.vector.tensor_copy(dst, trp[:, 0:RT])
                yield
            # 3 layers, per out-chunk psum [128, 400]
            h1 = dec.tile([128, 4, RT], F32R, tag='dh1', name=f'dh1_{ti}')
            h2 = dec.tile([128, 4, RT], F32R, tag='dh2', name=f'dh2_{ti}')
            for m in range(4):
                pt = psD.tile([128, RT], F32, tag='dl', name=f'dl1_{ti}_{m}')
                nc.tensor.matmul(pt, d1l[:, m, :], latT, start=True, stop=False)
                nc.tensor.matmul(pt, d1z[:, m, :], zT, start=False, stop=False)
                nc.tensor.matmul(pt, d1x[0:1, m * 128:(m + 1) * 128],
                                 xs[0:1, r0:r0 + RT], start=False, stop=True)
                yield
                if DVERELU:
                    nc.vector.tensor_scalar(h1[:, m, :], pt,
                                            dbias[:, m, 0:1], 0.0, ADD, MAX)
                else:
                    nc.scalar.activation(h1[:, m, :], pt, RELU,
                                         bias=dbias[:, m, 0:1])
            for m in range(4):
                pt = psD.tile([128, RT], F32, tag='dl', name=f'dl2_{ti}_{m}')
                for k in range(4):
                    nc.tensor.matmul(pt, d2[:, k, m, :], h1[:, k, :],
                                     start=(k == 0), stop=(k == 3))
                if DVERELU:
                    nc.vector.tensor_scalar(h2[:, m, :], pt,
                                            dbias[:, m, 1:2], 0.0, ADD, MAX)
                else:
                    nc.scalar.activation(h2[:, m, :], pt, RELU,
                                         bias=dbias[:, m, 1:2])
                yield
            o4 = dec.tile([128, 4, RT], F32, tag='o4', name=f'o4_{ti}') \
                if OMERGE else None
            for m in range(4):
                pt = psD.tile([128, RT], F32, tag='dl', name=f'dl3_{ti}_{m}')
                for k in range(4):
                    nc.tensor.matmul(pt, d3[:, k, m, :], h2[:, k, :],
                                     start=(k == 0), stop=(k == 3))
                if OMERGE:
                    nc.scalar.activation(o4[:, m, :], pt, RELU,
                                         bias=dbias[:, m, 2:3])
                    yield
                    continue
                ot = dec.tile([128, RT], F32, tag='ot', name=f'ot_{ti}_{m}')
                if DVERELU:
                    nc.vector.tensor_scalar(ot, pt, dbias[:, m, 2:3], 0.0,
                                            ADD, MAX)
                else:
                    nc.scalar.activation(ot, pt, RELU, bias=dbias[:, m, 2:3])
                nc.sync.dma_start(out=out_d[m][:, r0:r0 + RT], in_=ot)
                yield
            if OMERGE:
                oap = bass.AP(tensor=out_d.tensor, offset=out_d.offset + r0,
                              ap=[[ROWS, 128], [128 * ROWS, 4], [1, RT]])
                nc.sync.dma_start(out=oap, in_=o4)
                yield

        # ---- main schedule: ODE steps with decode tiles interleaved ----
        next_tile = 0
        drips = []
        for s in range(0 if not SKIP_ODE else NST, NST):
            kbs, parts = [], []
            emit_stage.h2sep = {}
            emit_stage.kse = {h: {} for h in range(NH)}
            for h in range(NH):
                if NH > 1:
                    hkp = psA.tile([128, 14 * NB], F32, tag=f'hk{h}',
                                   name=f'hk_{h}_{s}')
                else:
                    hkp = psA.tile([128, 4 * NB], F32, tag=f'hk{h}',
                                   name=f'hk_{h}_{s}')
                    kbt = psA.tile([128, 6 * NB], F32, tag=f'kb_{h}',
                                   name=f'kbt_{h}_{s}')
                    emit_stage.h2sep[h] = psA.tile(
                        [128, 4 * NB], F32, tag=f'h2_{h}', name=f'h2_{h}_{s}')

                def mk_kb(hkp=hkp, kbt=None if NH > 1 else kbt):
                    if NH > 1:
                        return lambda j: hkp[:, 4 * NB + j * NB:
                                             4 * NB + (j + 1) * NB]
                    return lambda j: kbt[:, j * NB:(j + 1) * NB]
                kbs.append((hkp, mk_kb()))
                parts.append({})
            def feed(k):
                for _ in range(k):
                    if not drips:
                        break
                    try:
                        next(drips[0])
                    except StopIteration:
                        drips.pop(0)
            if STEPORD:
                for h in range(NH):
                    for i in range(6):
                        emit_stage(h, s, i, kbs[h][0], kbs[h][1], parts[h])
                        emit_partials(h, s, i, kbs[h][1], parts[h])
                        if h == 0 and DRIP:
                            feed(DRIP)
            else:
                for i in range(6):
                    if DRIPPRE and DRIP:
                        feed(DRIP)
                    if HORDER:
                        for h in range(NH):
                            emit_stage(h, s, i, kbs[h][0], kbs[h][1], parts[h])
                            emit_partials(h, s, i, kbs[h][1], parts[h])
                            if DRIPFINE:
                                feed(1)
                    else:
                        for h in range(NH):
                            emit_stage(h, s, i, kbs[h][0], kbs[h][1], parts[h])
                            if DRIPFINE:
                                feed(1)
                        for h in range(NH):
                            emit_partials(h, s, i, kbs[h][1], parts[h])
                    if DRIP and not DRIPPRE:
                        feed(DRIP)
            for h in range(NH):
                emit_vupdate(h, s, kbs[h][1], parts[h])
            while next_tile < NTILE and maxinds[next_tile] <= s + 1:
                if not SKIP_DECODE:
                    if DRIP or DRIPFINE:
                        drips.append(emit_decode_gen(next_tile,
                                                     maxinds[next_tile]))
                    else:
                        emit_decode_tile(next_tile, maxinds[next_tile])
                next_tile += 1
        for g in drips:
            for _ in g:
                pass
        while next_tile < NTILE:
            if not SKIP_DECODE:
                emit_decode_tile(next_tile, maxinds[next_tile])
            next_tile += 1

    nc.compile()
    _BUILD_CACHE[key] = nc
    return nc


def _prep(x, z, initial_t, ode_W1, ode_b1, ode_W2, ode_b2, ode_W3, ode_b3,
          dec_W1, dec_b1, dec_W2, dec_b2, dec_W3, dec_b3):
    """All host-side preprocessing. Returns (in_maps, postprocess_info)."""
    x = np.asarray(x, np.float32)
    z = np.asarray(z, np.float32)
    initial_t = np.asarray(initial_t, np.float32)
    ode_W1 = np.asarray(ode_W1, np.float32)
    ode_b1 = np.asarray(ode_b1, np.float32)
    ode_W2 = np.asarray(ode_W2, np.float32)
    ode_b2 = np.asarray(ode_b2, np.float32)
    ode_W3 = np.asarray(ode_W3, np.float32)
    ode_b3 = np.asarray(ode_b3, np.float32)
    dec_W1 = np.asarray(dec_W1, np.float32)
    dec_b1 = np.asarray(dec_b1, np.float32)
    dec_W2 = np.asarray(dec_W2, np.float32)
    dec_b2 = np.asarray(dec_b2, np.float32)
    dec_W3 = np.asarray(dec_W3, np.float32)
    dec_b3 = np.asarray(dec_b3, np.float32)
    x0 = np.float32(np.asarray(initial_t).reshape(-1)[0])
    xi = x.reshape(B, N)
    xsort = np.concatenate([np.full((B, 1), x0, np.float32), xi], axis=1)
    times, inv = np.unique(xsort, return_inverse=True)
    assert times.size == NST + 1, f'unique times {times.size} != {NST + 1}'
    ind = inv.reshape(B, N + 1)[:, 1:].astype(np.int64)   # [B, N] in [1,100]
    assert ind.min() >= 1
    dts = (times[1:] - times[:-1]).astype(np.float32)

    # per-stage effective L1 bias (t-term + b3 feedthrough)
    w1t = ode_W1[ZD]                                   # [512]
    b3w1l = (ode_b3.astype(np.float64) @ ode_W1[:LD].astype(np.float64))
    b1eff = np.zeros((NST, 6, HD), np.float32)
    for s in range(NST):
        for i in range(6):
            t_si = np.float32(times[s]) + np.float32(dts[s]) * np.float32(C_[i])
            sa = float(np.sum([np.float32(dts[s]) * np.float32(a)
                               for a in A_[i]])) if A_[i] else 0.0
            b1eff[s, i] = (ode_b1.astype(np.float64) + float(t_si) *
                           w1t.astype(np.float64) + sa * b3w1l)
    sb = float(np.sum([np.float32(b) for b in B_]))
    # fold dt*coef into prescaled W3 variants; consumers use ratios
    fold = [A_[j + 1][j] for j in range(5)] + [B_[5]]
    vals, counts = np.unique(dts, return_counts=True)
    dt_nom = float(vals[np.argmax(counts)])
    cvals = [float(np.float32(dt_nom) * np.float32(fold[i])) for i in range(6)]
    _build.nv = 6
    _build.vmap = [i for s_ in range(NST) for i in range(6)]
    # per-step exact ratios: coefficient r*cv_j == fl32(dt_s)*fl32(a)
    ratio = [[[0.0] * 6 for _ in range(6)] for _ in range(NST)]
    ratiou = [[0.0] * 5 for _ in range(NST)]
    for s_ in range(NST):
        for i in range(1, 6):
            for j, a in enumerate(A_[i]):
                ratio[s_][i][j] = float(
                    np.float64(np.float32(dts[s_]) * np.float32(a))
                    / np.float64(cvals[j]))
        for j in range(5):
            if B_[j] != 0.0:
                ratiou[s_][j] = float(
                    np.float64(np.float32(dts[s_]) * np.float32(B_[j]))
                    / np.float64(cvals[j]))
    _build.ratio = ratio
    _build.ratiou = ratiou
    db3 = np.outer(dts.astype(np.float64) * sb,
                   ode_b3.astype(np.float64)).astype(np.float32)  # [NST, 128]

    def lhsT_tiles(w, kt, mt):
        # w [kt*128, mt*128] -> [128, kt, mt, 128]
        return np.ascontiguousarray(
            w.reshape(kt, 128, mt, 128).transpose(1, 0, 2, 3)).astype(np.float32)

    w1l_h = lhsT_tiles(ode_W1[:128], 1, 4).reshape(128, 4, 128)
    w2_h = lhsT_tiles(ode_W2, 4, 4)
    w3v_h = np.stack([lhsT_tiles(ode_W3 * np.float32(cv), 4, 1)
                      .reshape(128, 4, 128) for cv in cvals], axis=1)
    import ml_dtypes
    if BF16:
        w2_h = w2_h.astype(ml_dtypes.bfloat16)
    if BF16L3:
        w3v_h = w3v_h.astype(ml_dtypes.bfloat16)
    d1l_h = lhsT_tiles(dec_W1[1:129], 1, 4).reshape(128, 4, 128)
    d1z_h = lhsT_tiles(dec_W1[129:257], 1, 4).reshape(128, 4, 128)
    d1x_h = dec_W1[0:1].astype(np.float32)             # [1, 512]
    d2_h = lhsT_tiles(dec_W2, 4, 4)
    d3_h = lhsT_tiles(dec_W3, 4, 4)
    dbias_h = np.stack([dec_b1.reshape(4, 128).T, dec_b2.reshape(4, 128).T,
                        dec_b3.reshape(4, 128).T], axis=2).astype(np.float32)

    b1mm_h = np.zeros((NST * 6, 4, 512), np.float32)
    for cc in range(4):
        b1mm_h[:, cc, cc * 128:(cc + 1) * 128] = \
            b1eff.reshape(NST * 6, 512)[:, cc * 128:(cc + 1) * 128]
    b2mm_h = np.ascontiguousarray(ode_b2.reshape(4, 128)).astype(np.float32)
    onesel_h = np.zeros((4, 4, BL), np.float32)
    for cc in range(4):
        onesel_h[cc, cc, :] = 1.0
    onesid_h = np.zeros((68, 4, BL), np.float32)
    for cc in range(4):
        onesid_h[0:64, cc, :] = np.eye(64, dtype=np.float32)
        onesid_h[64 + cc, cc, :] = 1.0
    db3_h = np.ascontiguousarray(db3.T).astype(np.float32)  # [128, NST]

    # z static decode contribution is via gather; c1 is the ODE one

    in_maps = []
    tiles_info = []
    for c in range(NCORE):
        sl = slice(c * BL, (c + 1) * BL)
        zc = z[sl]
        ind_c = ind[sl].reshape(-1)                     # [12800]
        order = np.argsort(ind_c, kind='stable')
        ind_sorted = ind_c[order]
        b_sorted = (order // N).astype(np.int64)
        gvals = (ind_sorted * BL + b_sorted).astype(np.int16)
        gidx_h = np.zeros((128, NTILE * 32), np.int16)
        maxind_c = []
        for ti in range(NTILE):
            seg = gvals[ti * RT:(ti + 1) * RT]
            for j in range(RT):
                gidx_h[j % 16::16, ti * 32 + j // 16] = seg[j]
            maxind_c.append(int(ind_sorted[ti * RT:(ti + 1) * RT].max()))
        xs_h = xi[sl].reshape(-1)[order].astype(np.float32)[None, :]
        c1bm = (zc[:, LD:].astype(np.float64)
                @ ode_W1[LD:ZD].astype(np.float64)).astype(np.float32)
        in_maps.append({
            'w1l': w1l_h, 'w2': w2_h, 'w3v': w3v_h, 'c1bm': c1bm,
            'b1mm': b1mm_h, 'b2mm': b2mm_h, 'onesel': onesel_h,
            'onesid': onesid_h, 'db3': db3_h,
            'vlt0': np.ascontiguousarray(zc[:, :LD].T).astype(np.float32),
            'zl': np.ascontiguousarray(zc[:, :LD]).astype(np.float32),
            'zz': np.ascontiguousarray(zc[:, LD:]).astype(np.float32),
            'd1l': d1l_h, 'd1z': d1z_h, 'd1x': d1x_h, 'd2': d2_h, 'd3': d3_h,
            'dbias': dbias_h, 'xs': xs_h, 'gidx': gidx_h,
        })
        tiles_info.append((order, maxind_c))
    # all cores must share one schedule: use elementwise max over cores
    maxinds = [max(tiles_info[c][1][t] for c in range(NCORE))
               for t in range(NTILE)]
    orders = [tiles_info[c][0] for c in range(NCORE)]
    return in_maps, maxinds, orders, [float(d) for d in dts]


def _postprocess(results, orders):
    out = np.empty((B, N, HD), np.float32)
    for c in range(NCORE):
        o = results[c]['outT']                          # [4, 128, ROWS]
        flat = np.ascontiguousarray(o.transpose(2, 0, 1)).reshape(ROWS, HD)
        unsorted = np.empty_like(flat)
        unsorted[orders[c]] = flat
        out[c * BL:(c + 1) * BL] = unsorted.reshape(BL, N, HD)
    return out


def kernel(**inputs):
    in_maps, maxinds, orders, dts = _prep(**inputs)
    nc = _build(maxinds, dts)
    from concourse.bass_utils import run_bass_kernel_spmd
    res = run_bass_kernel_spmd(nc, in_maps, list(range(NCORE)))
    return _postprocess(res.results, orders)

